# revision 8
# baseline (speedup 1.0000x reference)
"""Trainium2 Bass kernel for a dense transformer block:
x -> LN1 -> causal MHA (16 heads) -> +residual -> LN2 -> FFN(4x, relu) -> +residual

Full inputs in, full outputs out. Sharding: 8 cores = (batch b in 0..3) x (parity p in 0..1).
Core (b, p) owns query blocks {2j+p : j in 0..3} of 256 tokens of batch b (1024 tokens),
computes K/V for the whole batch (duplicated within the pair), runs block-causal attention
with a uniform SPMD program (per-core causal masks passed as data), then proj/LN2/FFN on its
own token rows. No collectives. Matmuls in float32r (TF32-like, 1 cyc/row at free-dim>=256).

Layout notes:
 - "T" suffix = transposed layout [feature, token]; LN is computed in transposed layout with
   per-token stats built via ones-matmuls, broadcast across partitions on GpSimd.
 - LN applies in-place (xT tile becomes hT) to save SBUF; big slabs share pool tags so later
   phases (FFN uT) reuse attention-phase SBUF.
 - x2 (post-attention residual) is spilled to DRAM and re-read by LN2/FFN.
"""

import numpy as np

B, T, D = 4, 2048, 1024
H, DH = 16, 64
NG = 8            # head groups of 2 heads
TC = 1024         # tokens per core
QB = 256          # query block
NJ = 4            # local query blocks per core
F4 = 4096
EPS = 1e-5
SCALE = float(D) ** -0.5
N_CORES = 8

_cache = {}


def _build():
    import contextlib
    import concourse.bass as bass
    import concourse.mybir as mybir
    import concourse.tile as tile
    from concourse import bacc
    from concourse.masks import make_identity

    f32, f32r = mybir.dt.float32, mybir.dt.float32r
    AF = mybir.ActivationFunctionType
    ALU = mybir.AluOpType

    nc = bacc.Bacc('TRN2', target_bir_lowering=False, debug=False,
                   num_devices=N_CORES)

    # ---- external I/O (per-core) ----
    xT_d = nc.dram_tensor("xT", [D, T], f32, kind="ExternalInput")
    xqT_d = nc.dram_tensor("xqT", [D, TC], f32, kind="ExternalInput")
    xq_d = nc.dram_tensor("xq", [TC, D], f32, kind="ExternalInput")
    wq_d = nc.dram_tensor("wqp", [NG, 8, 128, 128], f32, kind="ExternalInput")
    wk_d = nc.dram_tensor("wkp", [NG, 8, 128, 128], f32, kind="ExternalInput")
    wv_d = nc.dram_tensor("wvp", [NG, 8, 128, 128], f32, kind="ExternalInput")
    wp_d = nc.dram_tensor("wp", [D, D], f32, kind="ExternalInput")
    w1_d = nc.dram_tensor("w1p", [32, 8, 128, 128], f32, kind="ExternalInput")
    w2_d = nc.dram_tensor("w2", [F4, D], f32, kind="ExternalInput")
    g1_d = nc.dram_tensor("g1t", [8, 128], f32, kind="ExternalInput")
    be1_d = nc.dram_tensor("be1t", [8, 128], f32, kind="ExternalInput")
    g2_d = nc.dram_tensor("g2t", [8, 128], f32, kind="ExternalInput")
    be2_d = nc.dram_tensor("be2t", [8, 128], f32, kind="ExternalInput")
    b1_d = nc.dram_tensor("b1t", [32, 128], f32, kind="ExternalInput")
    bp_d = nc.dram_tensor("bp", [D], f32, kind="ExternalInput")
    b2_d = nc.dram_tensor("b2", [D], f32, kind="ExternalInput")
    mk_d = nc.dram_tensor("masks", [4, 128, QB], f32, kind="ExternalInput")
    out_d = nc.dram_tensor("out", [TC, D], f32, kind="ExternalOutput")

    x2_dram = nc.dram_tensor("x2_scratch", [TC, D], f32)

    def bcast_ap(dram_ap, parts, n):
        return bass.AP(tensor=dram_ap.tensor, offset=dram_ap.offset,
                       ap=[[0, parts], [1, n]])

    with tile.TileContext(nc) as tc:
        ctx = contextlib.ExitStack()
        with ctx:
            consts = ctx.enter_context(tc.tile_pool(name="consts", bufs=1))
            persist = ctx.enter_context(tc.tile_pool(name="persist", bufs=1))
            # ---------- constants ----------
            g1t = consts.tile([128, 8], f32)
            nc.sync.dma_start(out=g1t, in_=g1_d[:, :].rearrange("g p -> p g"))
            be1t = consts.tile([128, 8], f32)
            nc.sync.dma_start(out=be1t, in_=be1_d[:, :].rearrange("g p -> p g"))
            g2t = consts.tile([128, 8], f32)
            nc.sync.dma_start(out=g2t, in_=g2_d[:, :].rearrange("g p -> p g"))
            be2t = consts.tile([128, 8], f32)
            nc.sync.dma_start(out=be2t, in_=be2_d[:, :].rearrange("g p -> p g"))
            ident_f = consts.tile([128, 128], f32)
            make_identity(nc, ident_f)
            ident_r = consts.tile([128, 128], f32r)
            nc.vector.tensor_copy(ident_r, ident_f)
            ones_f = consts.tile([128, 16], f32)
            nc.vector.memset(ones_f, 1.0)
            ones_r = consts.tile([128, 1], f32r)
            nc.vector.tensor_copy(ones_r, ones_f[:, 0:1])
            eps_t = consts.tile([1, 1], f32)
            nc.vector.memset(eps_t, EPS)

            # ---------- transposed layernorm (in-place capable) ----------
            def ln_T(src_all, dst_all, n_tok, gt, bt, wpool, spool, pspool):
                nch = n_tok // 512
                for c in range(nch):
                    sl = bass.ds(c * 512, 512)
                    mu_ps = pspool.tile([1, 512], f32, tag="mu_ps")
                    sq_ps = pspool.tile([1, 512], f32, tag="sq_ps")
                    for i in range(8):
                        sq = wpool.tile([128, 512], f32r, tag="sq")
                        nc.scalar.activation(out=sq, in_=src_all[:, i, sl].bitcast(f32),
                                             func=AF.Square)
                        nc.tensor.matmul(mu_ps, ones_r, src_all[:, i, sl],
                                         start=(i == 0), stop=(i == 7))
                        nc.tensor.matmul(sq_ps, ones_r, sq,
                                         start=(i == 0), stop=(i == 7))
                    mu = spool.tile([1, 512], f32, tag="mu")
                    nc.scalar.mul(mu, mu_ps, 1.0 / D)
                    sb = spool.tile([1, 512], f32, tag="sb")
                    nc.scalar.mul(sb, sq_ps, 1.0 / D)
                    sc = spool.tile([1, 512], f32, tag="sc")
                    nc.vector.tensor_tensor(out=sc, in0=mu, in1=mu, op=ALU.mult)
                    nc.vector.tensor_tensor(out=sb, in0=sb, in1=sc, op=ALU.subtract)
                    nc.scalar.activation(out=sb, in_=sb, func=AF.Sqrt, bias=eps_t)
                    nc.vector.reciprocal(sc, sb)
                    sd_ = spool.tile([1, 512], f32, tag="sd")
                    nc.vector.tensor_tensor(out=sd_, in0=sb, in1=sc, op=ALU.mult)
                    nc.vector.tensor_scalar(out=sd_, in0=sd_, scalar1=-1.0,
                                            scalar2=2.0, op0=ALU.mult, op1=ALU.add)
                    nc.vector.tensor_tensor(out=sd_, in0=sc, in1=sd_, op=ALU.mult)
                    mu_b = wpool.tile([128, 512], f32, tag="mu_b")
                    nc.gpsimd.partition_broadcast(mu_b, mu)
                    rstd_b = wpool.tile([128, 512], f32, tag="rstd_b")
                    nc.gpsimd.partition_broadcast(rstd_b, sd_)
                    for i in range(8):
                        t1 = wpool.tile([128, 512], f32, tag="t1")
                        nc.vector.tensor_tensor(out=t1,
                                                in0=src_all[:, i, sl].bitcast(f32),
                                                in1=mu_b, op=ALU.subtract)
                        nc.vector.tensor_tensor(out=t1, in0=t1, in1=rstd_b,
                                                op=ALU.mult)
                        nc.vector.tensor_scalar(out=dst_all[:, i, sl], in0=t1,
                                                scalar1=gt[:, i:i + 1],
                                                scalar2=bt[:, i:i + 1],
                                                op0=ALU.mult, op1=ALU.add)

            # ---------- LN1: xT -> hT (in place), xqT -> hqT (in place) ----------
            # persist tags: t64: hT -> uT(x2) ; t32a: hqT -> x2T ; t32b: attT
            hT = persist.tile([128, 8, T], f32r, tag="t64")
            hqT = persist.tile([128, 8, TC], f32r, tag="t32a")
            with tc.tile_pool(name="ln_work", bufs=3) as lnw, \
                 tc.tile_pool(name="ln_stats", bufs=1) as lns, \
                 tc.tile_pool(name="ps_ln1", bufs=1, space="PSUM") as ps_ln1:
                for i in range(8):
                    nc.sync.dma_start(
                        out=hT[:, i, :],
                        in_=xT_d[i * 128:(i + 1) * 128, :].bitcast(f32r))
                ln_T(hT, hT, T, g1t, be1t, lnw, lns, ps_ln1)
                for i in range(8):
                    nc.sync.dma_start(
                        out=hqT[:, i, :],
                        in_=xqT_d[i * 128:(i + 1) * 128, :].bitcast(f32r))
                ln_T(hqT, hqT, TC, g1t, be1t, lnw, lns, ps_ln1)

            # ---------- attention ----------
            attT = persist.tile([128, 8, TC], f32r, tag="t32b")
            with tc.tile_pool(name="kv_pool", bufs=1) as kvp, \
                 tc.tile_pool(name="wg_pool", bufs=1) as wgp, \
                 tc.tile_pool(name="att_work", bufs=4) as atw, \
                 tc.tile_pool(name="den_pool", bufs=1) as dnp, \
                 tc.tile_pool(name="ps_qkv", bufs=2, space="PSUM") as ps_qkv, \
                 tc.tile_pool(name="ps_st", bufs=2, space="PSUM") as ps_st, \
                 tc.tile_pool(name="ps_acc", bufs=2, space="PSUM") as ps_acc:
                mask_t = consts.tile([128, 4, QB], f32)
                nc.sync.dma_start(out=mask_t,
                                  in_=mk_d[:, :, :].rearrange("r p q -> p r q"))
                for g in range(NG):
                    wqg = wgp.tile([128, 8, 128], f32r, tag="wqg")
                    nc.sync.dma_start(
                        out=wqg, in_=wq_d[g].rearrange("k p c -> p k c").bitcast(f32r))
                    wkg = wgp.tile([128, 8, 128], f32r, tag="wkg")
                    nc.sync.dma_start(
                        out=wkg, in_=wk_d[g].rearrange("k p c -> p k c").bitcast(f32r))
                    wvg = wgp.tile([128, 8, 128], f32r, tag="wvg")
                    nc.sync.dma_start(
                        out=wvg, in_=wv_d[g].rearrange("k p c -> p k c").bitcast(f32r))
                    kt = kvp.tile([128, T], f32r, tag="kt")
                    vt = kvp.tile([128, T], f32r, tag="vt")
                    qt = kvp.tile([128, TC], f32r, tag="qt")
                    for n in range(4):
                        sl = bass.ds(n * 512, 512)
                        psk = ps_qkv.tile([128, 512], f32, tag="qkv")
                        for k in range(8):
                            nc.tensor.matmul(psk, wkg[:, k, :], hT[:, k, sl],
                                             start=(k == 0), stop=(k == 7))
                        nc.scalar.copy(kt[:, sl], psk)
                        psv = ps_qkv.tile([128, 512], f32, tag="qkv")
                        for k in range(8):
                            nc.tensor.matmul(psv, wvg[:, k, :], hT[:, k, sl],
                                             start=(k == 0), stop=(k == 7))
                        nc.scalar.copy(vt[:, sl], psv)
                    for n in range(2):
                        sl = bass.ds(n * 512, 512)
                        psq = ps_qkv.tile([128, 512], f32, tag="qkv")
                        for k in range(8):
                            nc.tensor.matmul(psq, wqg[:, k, :], hqT[:, k, sl],
                                             start=(k == 0), stop=(k == 7))
                        nc.scalar.copy(qt[:, sl], psq)
                    # V natural per head, with ones column (col 64)
                    vxa = kvp.tile([128, 16, 65], f32r, tag="vxa")
                    vxb = kvp.tile([128, 16, 65], f32r, tag="vxb")
                    nc.vector.tensor_copy(vxa[:, :, 64:65], ones_f.bitcast(f32r))
                    nc.vector.tensor_copy(vxb[:, :, 64:65], ones_f.bitcast(f32r))
                    for kt_i in range(16):
                        pst = ps_acc.tile([128, 128], f32r, tag="tr")
                        nc.tensor.transpose(pst, vt[:, kt_i * 128:(kt_i + 1) * 128],
                                            ident_r)
                        nc.vector.tensor_copy(vxa[:, kt_i, 0:64], pst[:, 0:64])
                        nc.vector.tensor_copy(vxb[:, kt_i, 0:64], pst[:, 64:128])
                    for h in range(2):
                        vx = vxa if h == 0 else vxb
                        hs = bass.ds(h * 64, 64)
                        den = dnp.tile([1, TC], f32, tag="den")
                        for j in range(NJ):
                            nkb = 4 * j + 4
                            acc = ps_acc.tile([65, QB], f32, tag="acc")
                            qsl = bass.ds(j * QB, QB)
                            for kb in range(nkb):
                                st = ps_st.tile([128, QB], f32, tag="st")
                                nc.tensor.matmul(
                                    st, kt[hs, kb * 128:(kb + 1) * 128],
                                    qt[hs, qsl], start=True, stop=True,
                                    tile_position=(h * 64, 0))
                                rel = kb - 4 * j
                                pt = atw.tile([128, QB], f32r, tag="pt")
                                if rel < 0:
                                    nc.scalar.activation(out=pt, in_=st,
                                                         func=AF.Exp, scale=SCALE)
                                else:
                                    ptm = atw.tile([128, QB], f32, tag="ptm")
                                    nc.scalar.activation(out=ptm, in_=st,
                                                         func=AF.Exp, scale=SCALE)
                                    nc.vector.tensor_tensor(
                                        out=pt, in0=ptm, in1=mask_t[:, rel, :],
                                        op=ALU.mult)
                                nc.tensor.matmul(acc, vx[:, kb, :], pt,
                                                 start=(kb == 0),
                                                 stop=(kb == nkb - 1))
                            nc.scalar.copy(attT[hs, g, qsl], acc[0:64, :])
                            nc.scalar.copy(den[0:1, qsl], acc[64:65, :])
                        rden = dnp.tile([1, TC], f32, tag="rden")
                        nc.vector.reciprocal(rden, den)
                        rb = dnp.tile([128, TC], f32, tag="rb")
                        nc.gpsimd.partition_broadcast(rb, rden)
                        nc.vector.tensor_tensor(out=attT[hs, g, :],
                                                in0=attT[hs, g, :].bitcast(f32),
                                                in1=rb[hs, :], op=ALU.mult)

            # ---------- proj + residual -> x2 (spilled to DRAM) ----------
            with tc.tile_pool(name="proj_pool", bufs=2) as prp, \
                 tc.tile_pool(name="proj_c", bufs=1) as prc, \
                 tc.tile_pool(name="ps_proj", bufs=2, space="PSUM") as ps_proj:
                bp_b = prc.tile([128, D], f32)
                nc.sync.dma_start(out=bp_b, in_=bcast_ap(bp_d[:], 128, D))
                wp_sb = prc.tile([128, 8, D], f32r)
                nc.sync.dma_start(
                    out=wp_sb,
                    in_=wp_d[:, :].rearrange("(k p) o -> p k o", p=128).bitcast(f32r))
                for mt in range(8):
                    xqt = prp.tile([128, D], f32, tag="xq")
                    nc.sync.dma_start(out=xqt, in_=xq_d[mt * 128:(mt + 1) * 128, :])
                    for oc in range(2):
                        osl = bass.ds(oc * 512, 512)
                        ps = ps_proj.tile([128, 512], f32, tag="proj")
                        for k in range(8):
                            nc.tensor.matmul(ps, attT[:, k, mt * 128:(mt + 1) * 128],
                                             wp_sb[:, k, osl],
                                             start=(k == 0), stop=(k == 7))
                        tt = prp.tile([128, 512], f32, tag="tt")
                        nc.vector.tensor_tensor(out=tt, in0=ps, in1=xqt[:, osl],
                                                op=ALU.add)
                        nc.vector.tensor_tensor(out=tt, in0=tt, in1=bp_b[:, osl],
                                                op=ALU.add)
                        nc.sync.dma_start(
                            out=x2_dram[mt * 128:(mt + 1) * 128,
                                        oc * 512:(oc + 1) * 512],
                            in_=tt)

            # ---------- LN2 (transposed) + FFN per 512-token chunk ----------
            with tc.tile_pool(name="ffn_c", bufs=1) as fcc, \
                 tc.tile_pool(name="ffn_x", bufs=2) as fx, \
                 tc.tile_pool(name="ffn_w", bufs=3) as fw, \
                 tc.tile_pool(name="ln2_work", bufs=3) as lnw2, \
                 tc.tile_pool(name="ln2_stats", bufs=1) as lns2, \
                 tc.tile_pool(name="ps_ln2", bufs=1, space="PSUM") as ps_ln2, \
                 tc.tile_pool(name="ps_u", bufs=2, space="PSUM") as ps_u, \
                 tc.tile_pool(name="ps_v", bufs=1, space="PSUM") as ps_v:
                b1t = fcc.tile([128, 32], f32)
                nc.sync.dma_start(out=b1t, in_=b1_d[:, :].rearrange("i p -> p i"))
                b2_b = fcc.tile([128, D], f32)
                nc.sync.dma_start(out=b2_b, in_=bcast_ap(b2_d[:], 128, D))
                for tcx in range(2):
                    x2T = persist.tile([128, 8, 512], f32r, tag="t32a")
                    for mtl in range(4):
                        x2ld = fx.tile([128, D], f32, tag="x2ld")
                        mt = tcx * 4 + mtl
                        nc.sync.dma_start(out=x2ld,
                                          in_=x2_dram[mt * 128:(mt + 1) * 128, :])
                        for i in range(8):
                            pst = ps_u.tile([128, 128], f32, tag="u")
                            nc.tensor.transpose(pst, x2ld[:, i * 128:(i + 1) * 128],
                                                ident_f)
                            nc.vector.tensor_copy(
                                x2T[:, i, mtl * 128:(mtl + 1) * 128], pst)
                    ln_T(x2T, x2T, 512, g2t, be2t, lnw2, lns2, ps_ln2)
                    uT = persist.tile([128, 32, 512], f32r, tag="t64")
                    for i in range(32):
                        w1t = fw.tile([128, 8, 128], f32r, tag="w1t")
                        nc.sync.dma_start(
                            out=w1t,
                            in_=w1_d[i].rearrange("k p c -> p k c").bitcast(f32r))
                        psu = ps_u.tile([128, 512], f32, tag="u")
                        for k in range(8):
                            nc.tensor.matmul(psu, w1t[:, k, :], x2T[:, k, :],
                                             start=(k == 0), stop=(k == 7))
                        nc.scalar.activation(out=uT[:, i, :], in_=psu, func=AF.Relu,
                                             bias=b1t[:, i:i + 1])
                    for oc in range(2):
                        osl = bass.ds(oc * 512, 512)
                        psv = []
                        for mtl in range(4):
                            psv_t = ps_v.tile([128, 512], f32, tag=f"v{mtl}")
                            psv.append(psv_t)
                        for i in range(32):
                            w2t = fw.tile([128, 512], f32r, tag="w2t")
                            nc.sync.dma_start(
                                out=w2t,
                                in_=w2_d[i * 128:(i + 1) * 128,
                                         oc * 512:(oc + 1) * 512].bitcast(f32r))
                            for mtl in range(4):
                                nc.tensor.matmul(
                                    psv[mtl], uT[:, i, mtl * 128:(mtl + 1) * 128],
                                    w2t, start=(i == 0), stop=(i == 31))
                        for mtl in range(4):
                            mt = tcx * 4 + mtl
                            x2r = fx.tile([128, 512], f32, tag="x2r")
                            nc.sync.dma_start(
                                out=x2r,
                                in_=x2_dram[mt * 128:(mt + 1) * 128,
                                            oc * 512:(oc + 1) * 512])
                            ot = fx.tile([128, 512], f32, tag="ot")
                            nc.vector.tensor_tensor(out=ot, in0=psv[mtl],
                                                    in1=b2_b[:, osl], op=ALU.add)
                            nc.vector.tensor_tensor(out=ot, in0=ot, in1=x2r,
                                                    op=ALU.add)
                            nc.sync.dma_start(
                                out=out_d[mt * 128:(mt + 1) * 128,
                                          oc * 512:(oc + 1) * 512],
                                in_=ot)

    nc.compile()
    return nc


def _prep_shared(wq, wk, wv, wp, bp, w1, b1, w2, b2, g1, be1, g2, be2):
    c = np.ascontiguousarray
    f = np.float32

    def cf(a):
        return c(np.asarray(a, f))

    return {
        "wqp": c(np.asarray(wq, f).reshape(8, 128, 8, 128).transpose(2, 0, 1, 3)),
        "wkp": c(np.asarray(wk, f).reshape(8, 128, 8, 128).transpose(2, 0, 1, 3)),
        "wvp": c(np.asarray(wv, f).reshape(8, 128, 8, 128).transpose(2, 0, 1, 3)),
        "wp": cf(wp),
        "w1p": c(np.asarray(w1, f).reshape(8, 128, 32, 128).transpose(2, 0, 1, 3)),
        "w2": cf(w2),
        "g1t": cf(g1).reshape(8, 128),
        "be1t": cf(be1).reshape(8, 128),
        "g2t": cf(g2).reshape(8, 128),
        "be2t": cf(be2).reshape(8, 128),
        "b1t": cf(b1).reshape(32, 128),
        "bp": cf(bp),
        "b2": cf(b2),
    }


def _own_idx(p):
    return (np.arange(NJ)[:, None] * 512 + p * QB + np.arange(QB)[None, :]).ravel()


def _masks(p):
    m = np.zeros((4, 128, QB), np.float32)
    k = np.arange(128)[:, None]
    q = np.arange(QB)[None, :]
    for rel in range(4):
        m[rel] = (128 * rel + k <= QB * p + q).astype(np.float32)
    return m


def _make_in_maps(x, shared):
    in_maps = []
    for c in range(N_CORES):
        b, p = c // 2, c % 2
        xb = np.asarray(x[b], np.float32)
        idx = _own_idx(p)
        xq = np.ascontiguousarray(xb[idx])
        m = dict(shared)
        m["xT"] = np.ascontiguousarray(xb.T)
        m["xq"] = xq
        m["xqT"] = np.ascontiguousarray(xq.T)
        m["masks"] = _masks(p)
        in_maps.append(m)
    return in_maps


def kernel(**inputs):
    from concourse.bass_utils import run_bass_kernel_spmd

    if "nc" not in _cache:
        _cache["nc"] = _build()
    nc = _cache["nc"]

    shared = _prep_shared(
        inputs["wq"], inputs["wk"], inputs["wv"], inputs["wp"], inputs["bp"],
        inputs["w1"], inputs["b1"], inputs["w2"], inputs["b2"],
        inputs["g1"], inputs["be1"], inputs["g2"], inputs["be2"])
    in_maps = _make_in_maps(inputs["x"], shared)

    res = run_bass_kernel_spmd(nc, in_maps, list(range(N_CORES)))
    out = np.empty((B, T, D), np.float32)
    for c in range(N_CORES):
        b, p = c // 2, c % 2
        out[b][_own_idx(p)] = res.results[c]["out"]
    return out
